# revision 35
# baseline (speedup 1.0000x reference)
"""Multi-head attention forward (B=2, S=2048, E=1024, H=16, D=64) on 8 TRN2
NeuronCores, tensor-parallel across heads (2 heads/core).

Per core: QKV^T projection with X^T streamed as the moving operand, attention
computed in the S^T/attn^T orientation (softmax denominator obtained by
appending a ones column to V in the PV matmul), out-projection of the core's
128 embed dims giving a partial [4096, 1024] output. Host sums the 8 partials
and adds the output bias.

Schedule: the attention inner loop is Activation-bound (exp), so all other
PE work is threaded through it: QKV row-blocks of batch b+1 and the
out-projection of the previous query block are emitted between score/PV
groups, attention(b0, qb0) starts as soon as the first QKV row block of b0
is done, and V transposes ride in the qkv/out-proj psum pool.
"""

import os
from contextlib import ExitStack

import numpy as np

import concourse.bass as bass
import concourse.mybir as mybir
import concourse.tile as tile
from concourse import bacc
from concourse.masks import make_identity

# ---- problem constants (hardcoded per contract) ----
B, S, E, H, D = 2, 2048, 1024, 16, 64
P = 128                      # partitions
R = B * S                    # 4096 flattened rows
KO = E // P                  # 8 contraction chunks over E
NKC = S // P                 # 16 key chunks per sequence
HC = 2                       # heads per core
NCORES = 8
RB = 512                     # row block for the QKV projection
NRB = S // RB                # row blocks per batch (4)

MM_MODE = os.environ.get("MHA_MM_MODE", "bf16")
QB_OVERRIDE = int(os.environ.get("MHA_QB", "0"))        # 0 = mode default
ES_BUFS = int(os.environ.get("MHA_ES_BUFS", "8"))
KCG = int(os.environ.get("MHA_KCG", "2"))               # kc per exp group
SC_BUFS = int(os.environ.get("MHA_SC_BUFS", "2"))
PQ_BUFS = int(os.environ.get("MHA_PQ_BUFS", "2"))
Y_BF16 = bool(int(os.environ.get("MHA_Y_BF16", "1")))

FP32 = mybir.dt.float32
BF16 = mybir.dt.bfloat16
EXP = mybir.ActivationFunctionType.Exp


def _mode_params(mm_mode):
    if mm_mode == "bf16":
        dt, qb = mybir.dt.bfloat16, 512
    elif mm_mode == "f32r":
        dt, qb = mybir.dt.float32r, 512
    elif mm_mode == "f32":
        dt, qb = FP32, 256
    else:
        raise ValueError(mm_mode)
    return dt, (QB_OVERRIDE or qb)


def build_kernel(tc, xt, wqkv, bqkv, wout, y, sdt, QB, mm_mode, ctx):
    nc = tc.nc
    NQB = S // QB
    NG = NKC // KCG
    ydt = BF16 if Y_BF16 else FP32

    # float32r can only be produced by rounding-capable engine ops (ACT/DVE
    # outputs) or DMA of host-pre-rounded data; memset/affine_select cannot.
    vdt = FP32 if sdt == mybir.dt.float32r else sdt

    const = ctx.enter_context(tc.tile_pool(name="const", bufs=1))
    # PSUM budget: 8 banks = scores 2x2 + qkv/transpose/out-proj 2x1 + pa 2x1
    ps_sc = ctx.enter_context(tc.tile_pool(name="ps_sc", bufs=SC_BUFS,
                                           space="PSUM"))
    ps_q = ctx.enter_context(tc.tile_pool(name="ps_q", bufs=PQ_BUFS, space="PSUM"))
    ps_pa = ctx.enter_context(tc.tile_pool(name="ps_pa", bufs=2, space="PSUM"))

    xt_pool = ctx.enter_context(tc.tile_pool(name="xtp", bufs=3))
    exps_pool = ctx.enter_context(tc.tile_pool(name="exps", bufs=ES_BUFS))
    rc_pool = ctx.enter_context(tc.tile_pool(name="rc", bufs=2))
    bc_pool = ctx.enter_context(tc.tile_pool(name="bc", bufs=2))
    y_pool = ctx.enter_context(tc.tile_pool(name="yp", bufs=4))

    wq_r = wqkv.rearrange("(ko p) m -> p ko m", p=P)
    xt_r = xt.rearrange("(ko p) r -> p ko r", p=P)

    wq_sb = const.tile([P, KO, 3 * P], sdt)
    bq_sb = const.tile([P, 3], FP32)
    wo_sb = const.tile([P, E], sdt)
    ident = const.tile([P, P], vdt)

    qt = const.tile([P, B, S], sdt)       # Q^T  [2h*64, b, s]
    kt = const.tile([P, B, S], sdt)       # K^T
    vt = const.tile([P, B, S], vdt)       # V^T
    v1 = const.tile([P, B, HC, NKC, D + 1], sdt)  # V natural + ones col
    attnT = const.tile([P, B, S], sdt)    # unnormalized-then-normalized attn^T

    # ---- initial loads: wq/xt interleaved in fine chunks so the first QKV
    # matmuls start as soon as their slices land ----
    xt0 = xt_pool.tile([P, KO, RB], sdt, tag="xt", name="xt0")
    for ko in range(0, KO, 2):
        nc.sync.dma_start(wq_sb[:, ko:ko + 2, :], wq_r[:, ko:ko + 2, :])
        nc.sync.dma_start(xt0[:, ko:ko + 2, :], xt_r[:, ko:ko + 2, 0:RB])
        if ko == 0:
            nc.sync.dma_start(bq_sb, bqkv.rearrange("(m p) -> p m", p=P))
    nc.sync.dma_start(wo_sb, wout)

    make_identity(nc, ident)
    ones_col = const.tile([P, 1], FP32)
    nc.vector.memset(ones_col, 1.0)
    nc.vector.tensor_copy(v1[:, :, :, :, D:],
                          ones_col.to_broadcast((P, B, HC, NKC, 1)))
    ones_row = const.tile([1, D], vdt)
    nc.vector.memset(ones_row, 1.0)

    xt_tiles = {(0, 0): xt0}

    def emit_xt_dma(b, rbi):
        rb = b * NRB + rbi
        t = xt_pool.tile([P, KO, RB], sdt, tag="xt", name=f"xt_{rb}")
        nc.sync.dma_start(t, xt_r[:, :, rb * RB:(rb + 1) * RB])
        xt_tiles[(b, rbi)] = t

    def emit_mchunk(b, rbi, m):
        """One QKV dest (q/k/v) for one row block: 8 matmuls + bias add."""
        col = rbi * RB
        dest = (qt, kt, vt)[m]
        xt_t = xt_tiles[(b, rbi)]
        pst = ps_q.tile([P, RB], FP32, tag="pq", name=f"ps_qkv_{b}_{rbi}_{m}")
        for ko in range(KO):
            nc.tensor.matmul(
                pst, wq_sb[:, ko, m * P:(m + 1) * P], xt_t[:, ko, :],
                start=(ko == 0), stop=(ko == KO - 1))
        nc.vector.tensor_scalar_add(dest[:, b, col:col + RB], pst,
                                    bq_sb[:, m:m + 1])

    def emit_tr(b, rbi):
        """V transposes for one row block (into the qkv psum pool)."""
        for kci in range(RB // P):
            kc = rbi * (RB // P) + kci
            pst = ps_q.tile([P, P], vdt, tag="pq", name=f"ps_tr_{b}_{kc}")
            nc.tensor.transpose(pst, vt[:, b, kc * P:(kc + 1) * P], ident)
            nc.vector.tensor_copy(
                v1[:, b, :, kc, 0:D],
                pst.rearrange("p (h d) -> p h d", h=HC))

    def emit_outproj(b, qb, use_act=False):
        for qc in range(QB // P):
            q0 = qb * QB + qc * P
            yt = y_pool.tile([P, E], ydt, tag="yt", name=f"yt_{b}_{qb}_{qc}")
            # on the final (drain) out-proj, alternate psum pools per qc to
            # double rotation depth; elsewhere ps_sc is owned by the scores
            pool, tg = (ps_q, "pq")
            if use_act and qc % 2 == 1:
                pool, tg = (ps_sc, "sc")
            for nh in range(2):
                pst = pool.tile([P, 512], FP32, tag=tg,
                                name=f"ps_y_{b}_{qb}_{qc}_{nh}")
                nc.tensor.matmul(
                    pst, attnT[:, b, q0:q0 + P],
                    wo_sb[:, nh * 512:(nh + 1) * 512],
                    start=True, stop=True)
                if use_act and nh == 1:
                    # drain path: ACT is idle, split the psum->sbuf copies
                    nc.scalar.copy(yt[:, nh * 512:(nh + 1) * 512], pst)
                else:
                    nc.vector.tensor_copy(yt[:, nh * 512:(nh + 1) * 512], pst)
            nc.sync.dma_start(y[b * S + q0: b * S + q0 + P, :], yt)

    def emit_attention(b, qb, pre_hooks=None, post=None, fast_norm=False,
                       last=False):
        pre_hooks = pre_hooks or {}
        pa = {}
        for h in range(HC):
            pa[h] = ps_pa.tile([P, QB], FP32, tag="pa",
                               name=f"pa_{b}_{qb}_{h}")
        def emit_pv(est, g):
            for h in range(HC):
                for j in range(KCG):
                    kc = g * KCG + j
                    nc.tensor.matmul(
                        pa[h][:D + 1, :], v1[:, b, h, kc, :],
                        est[h][:, j, :],
                        start=(kc == 0), stop=(kc == NKC - 1),
                        skip_group_check=True)

        # PV lags scores by one group so the first PV (which must wait for
        # the previous qb's normalize to release the pa bank) sits later in
        # the PE queue.
        pv_pending = None
        for g in range(NG):
            for fn in pre_hooks.get(g, ()):
                fn()
            est = {}
            pst = {}
            for h in range(HC):
                est[h] = exps_pool.tile([P, KCG, QB], sdt, tag="es",
                                        name=f"es_{b}_{qb}_{g}_{h}")
                pst[h] = ps_sc.tile([P, KCG, QB], FP32, tag="sc",
                                    name=f"ps_sc_{b}_{qb}_{g}_{h}")
            for h in range(HC):
                for j in range(KCG):
                    kc = g * KCG + j
                    nc.tensor.matmul(
                        pst[h][:, j, :],
                        kt[h * D:(h + 1) * D, b, kc * P:(kc + 1) * P],
                        qt[h * D:(h + 1) * D, b, qb * QB:(qb + 1) * QB],
                        start=True, stop=True)
            for h in range(HC):
                nc.scalar.activation(est[h], pst[h], EXP, scale=0.125)
            if pv_pending is not None:
                emit_pv(*pv_pending)
            pv_pending = (est, g)
        emit_pv(*pv_pending)
        for h in range(HC):
            if fast_norm:
                # drain path: broadcast 1/den across partitions with a tiny
                # PE matmul (ones [1,D] stationary) — shorter latency than
                # the gpsimd partition_broadcast, and PE is idle here
                rc = rc_pool.tile([1, QB], vdt, tag="rc",
                                  name=f"rc_{b}_{qb}_{h}")
                with nc.allow_low_precision("bf16 1/den on drain qb only"):
                    nc.vector.reciprocal(rc, pa[h][D:D + 1, :])
                bcp = ps_q.tile([D, QB], FP32, tag="pq",
                                name=f"bcp_{b}_{qb}_{h}")
                nc.tensor.matmul(bcp, ones_row, rc, start=True, stop=True)
                nc.vector.tensor_tensor(
                    attnT[h * D:(h + 1) * D, b, qb * QB:(qb + 1) * QB],
                    pa[h][:D, :], bcp, mybir.AluOpType.mult)
            elif not last:
                rc = rc_pool.tile([1, QB], FP32, tag="rc",
                                  name=f"rc_{b}_{qb}_{h}")
                nc.vector.reciprocal(rc, pa[h][D:D + 1, :])
                # stage pa out of PSUM right away so the bank frees for the
                # next qb's PV before the broadcast/mult chain finishes
                sc = bc_pool.tile([D, QB], FP32, tag="pascr",
                                  name=f"pasc_{b}_{qb}_{h}")
                nc.vector.tensor_copy(sc, pa[h][:D, :])
                bc = bc_pool.tile([D, QB], FP32, tag="bc",
                                  name=f"bc_{b}_{qb}_{h}")
                nc.gpsimd.partition_broadcast(bc, rc)
                nc.vector.tensor_tensor(
                    attnT[h * D:(h + 1) * D, b, qb * QB:(qb + 1) * QB],
                    sc, bc, mybir.AluOpType.mult)
            else:
                # final qb: no next PV to unblock — shortest chain to attnT
                rc = rc_pool.tile([1, QB], FP32, tag="rc",
                                  name=f"rc_{b}_{qb}_{h}")
                nc.vector.reciprocal(rc, pa[h][D:D + 1, :])
                bc = bc_pool.tile([D, QB], FP32, tag="bc",
                                  name=f"bc_{b}_{qb}_{h}")
                nc.gpsimd.partition_broadcast(bc, rc)
                nc.vector.tensor_tensor(
                    attnT[h * D:(h + 1) * D, b, qb * QB:(qb + 1) * QB],
                    pa[h][:D, :], bc, mybir.AluOpType.mult)
        if post:
            post()

    def rb_hook(b, rbi, with_tr_of=None, dma_next=None):
        """pre_hooks dict entries that weave one row block of QKV through an
        attention qb: m-chunks after groups 1/3/5, transposes after 6."""
        hooks = {}
        if dma_next is not None:
            hooks.setdefault(0, []).append(
                lambda bn=dma_next[0], rn=dma_next[1]: emit_xt_dma(bn, rn))
        hooks.setdefault(1, []).append(lambda: emit_mchunk(b, rbi, 0))
        hooks.setdefault(3, []).append(lambda: emit_mchunk(b, rbi, 1))
        hooks.setdefault(5, []).append(lambda: emit_mchunk(b, rbi, 2))
        if with_tr_of is not None:
            hooks.setdefault(6, []).append(
                lambda r=with_tr_of: emit_tr(b, r))
        return hooks

    # ---- segment 1: QKV(b0) row blocks interleaved with attention(b0, 0) ----
    # rb0 must be fully emitted before group 0 (kt/qt/v1 for kc 0..3)
    emit_xt_dma(0, 1)
    for m in range(3):
        emit_mchunk(0, 0, m)
    seg1_hooks = {}
    seg1_hooks[0] = [lambda: emit_tr(0, 0), lambda: emit_xt_dma(0, 2)]
    seg1_hooks[1] = [lambda: (emit_mchunk(0, 1, 0), emit_mchunk(0, 1, 1),
                              emit_mchunk(0, 1, 2), emit_tr(0, 1),
                              emit_xt_dma(0, 3))]
    seg1_hooks[3] = [lambda: (emit_mchunk(0, 2, 0), emit_mchunk(0, 2, 1),
                              emit_mchunk(0, 2, 2), emit_tr(0, 2))]
    seg1_hooks[5] = [lambda: (emit_mchunk(0, 3, 0), emit_mchunk(0, 3, 1),
                              emit_mchunk(0, 3, 2), emit_tr(0, 3),
                              emit_xt_dma(1, 0))]
    emit_attention(0, 0, pre_hooks=seg1_hooks)

    # ---- segment 2: attention(b0, 1..3) carrying QKV(b1) rb 0..2 ----
    for qb in range(1, NQB):
        r = qb - 1
        hooks = rb_hook(1, r, with_tr_of=(r - 1) if r >= 1 else None,
                        dma_next=(1, r + 1) if r + 1 < NRB else None)
        emit_attention(0, qb, pre_hooks=hooks,
                       post=lambda q=qb - 1: emit_outproj(0, q))

    # ---- segment 3: attention(b1, 0..3), finishing QKV(b1) rb3 ----
    hooks = {
        1: [lambda: emit_mchunk(1, 3, 0)],
        3: [lambda: emit_mchunk(1, 3, 1)],
        4: [lambda: emit_tr(1, 2)],
        5: [lambda: emit_mchunk(1, 3, 2)],
        6: [lambda: emit_tr(1, 3)],
    }
    emit_attention(1, 0, pre_hooks=hooks, post=lambda: emit_outproj(0, 3))
    for qb in range(1, NQB):
        emit_attention(1, qb, post=lambda q=qb - 1: emit_outproj(1, q),
                       last=(qb == NQB - 1))
    emit_outproj(1, NQB - 1, use_act=True)


def build_nc(mm_mode=MM_MODE, reps=1):
    sdt, QB = _mode_params(mm_mode)
    ydt = BF16 if Y_BF16 else FP32
    nc = bacc.Bacc("TRN2", target_bir_lowering=False, debug=False)
    xt = nc.dram_tensor("xt", [E, R], sdt, kind="ExternalInput").ap()
    wqkv = nc.dram_tensor("wqkv", [E, 3 * P], sdt, kind="ExternalInput").ap()
    bqkv = nc.dram_tensor("bqkv", [3 * P], FP32, kind="ExternalInput").ap()
    wout = nc.dram_tensor("wout", [P, E], sdt, kind="ExternalInput").ap()
    y = nc.dram_tensor("y", [R, E], ydt, kind="ExternalOutput").ap()
    with tile.TileContext(nc) as tc:
        for _ in range(reps):
            with ExitStack() as ctx:
                build_kernel(tc, xt, wqkv, bqkv, wout, y, sdt, QB, mm_mode, ctx)
    nc.compile()
    return nc


def _round_f32r(x):
    """Round fp32 to the fp32r grid (11 explicit mantissa bits) the way the
    hardware expects matmul operands: add-half then truncate the low 12 bits."""
    bits = np.ascontiguousarray(x, np.float32).view(np.uint32)
    return (((bits + np.uint32(0x800)) & np.uint32(0xFFFFF000))
            .view(np.float32))


def shard_inputs(input_tensor, qkv_w, qkv_b, out_w, mm_mode=MM_MODE):
    """Build the 8 per-core input maps (numpy, host-side)."""
    sdt, _ = _mode_params(mm_mode)
    np_sdt = mybir.dt.np(sdt)

    def prep(a):
        a = np.ascontiguousarray(a).astype(np_sdt)
        return _round_f32r(a) if mm_mode == "f32r" else a

    X = np.asarray(input_tensor, np.float32).reshape(R, E)
    XT = prep(X.T)
    qkv_w = np.asarray(qkv_w, np.float32)
    qkv_b = np.asarray(qkv_b, np.float32)
    out_w = np.asarray(out_w, np.float32)
    in_maps = []
    for c in range(NCORES):
        sl = slice(c * P, (c + 1) * P)
        wq = np.concatenate(
            [qkv_w[:, sl], qkv_w[:, E + c * P:E + (c + 1) * P],
             qkv_w[:, 2 * E + c * P:2 * E + (c + 1) * P]], axis=1)
        bq = np.concatenate(
            [qkv_b[sl], qkv_b[E + c * P:E + (c + 1) * P],
             qkv_b[2 * E + c * P:2 * E + (c + 1) * P]])
        in_maps.append({
            "xt": XT,
            "wqkv": prep(wq),
            "bqkv": np.ascontiguousarray(bq),
            "wout": prep(out_w[sl, :]),
        })
    return in_maps


_NC_CACHE = {}


def _get_nc(mm_mode):
    if mm_mode not in _NC_CACHE:
        _NC_CACHE[mm_mode] = build_nc(mm_mode)
    return _NC_CACHE[mm_mode]


LAST_RESULT = None


def kernel(input_tensor, qkv_w, qkv_b, out_w, out_b):
    global LAST_RESULT
    from concourse import bass_utils
    nc = _get_nc(MM_MODE)
    in_maps = shard_inputs(input_tensor, qkv_w, qkv_b, out_w, MM_MODE)
    res = bass_utils.run_bass_kernel_spmd(
        nc, in_maps, core_ids=list(range(NCORES)),
        trace=bool(int(os.environ.get("MHA_TRACE", "0"))))
    LAST_RESULT = res
    out = np.zeros((R, E), np.float32)
    for r in res.results:
        out += np.asarray(r["y"], np.float32)
    out += np.asarray(out_b, np.float32)
    return out.reshape(B, S, E)


def core_partial_ref(input_tensor, qkv_w, qkv_b, out_w, c):
    """Exact fp32 numpy reference for core c's partial output (for testing)."""
    X = np.asarray(input_tensor, np.float32).reshape(R, E)
    out = np.zeros((R, E), np.float32)
    for b in range(B):
        rows = slice(b * S, (b + 1) * S)
        for hl in range(HC):
            h = c * HC + hl
            q = X[rows] @ qkv_w[:, h * D:(h + 1) * D] + qkv_b[h * D:(h + 1) * D]
            k = X[rows] @ qkv_w[:, E + h * D:E + (h + 1) * D] + qkv_b[E + h * D:E + (h + 1) * D]
            v = X[rows] @ qkv_w[:, 2 * E + h * D:2 * E + (h + 1) * D] + qkv_b[2 * E + h * D:2 * E + (h + 1) * D]
            s = (q @ k.T) / np.sqrt(np.float32(D))
            p = np.exp(s - s.max(axis=1, keepdims=True))
            p /= p.sum(axis=1, keepdims=True)
            a = p @ v
            out[rows] += a @ out_w[h * D:(h + 1) * D, :]
    return out


# revision 37
# speedup vs baseline: 1.1289x; 1.1289x over previous
"""Multi-head attention forward (B=2, S=2048, E=1024, H=16, D=64) on 8 TRN2
NeuronCores, tensor-parallel across heads (2 heads/core).

Per core: QKV^T projection with X^T streamed as the moving operand, attention
computed in the S^T/attn^T orientation (softmax denominator obtained by
appending a ones column to V in the PV matmul), out-projection of the core's
128 embed dims giving a partial [4096, 1024] output. Host sums the 8 partials
and adds the output bias.

Schedule: the attention inner loop is Activation-bound (exp), so all other
PE work is threaded through it: QKV row-blocks of batch b+1 and the
out-projection of the previous query block are emitted between score/PV
groups, attention(b0, qb0) starts as soon as the first QKV row block of b0
is done, and V transposes ride in the qkv/out-proj psum pool.
"""

import os
from contextlib import ExitStack

import numpy as np

import concourse.bass as bass
import concourse.mybir as mybir
import concourse.tile as tile
from concourse import bacc
from concourse.masks import make_identity

# ---- problem constants (hardcoded per contract) ----
B, S, E, H, D = 2, 2048, 1024, 16, 64
P = 128                      # partitions
R = B * S                    # 4096 flattened rows
KO = E // P                  # 8 contraction chunks over E
NKC = S // P                 # 16 key chunks per sequence
HC = 2                       # heads per core
NCORES = 8
RB = 512                     # row block for the QKV projection
NRB = S // RB                # row blocks per batch (4)

MM_MODE = os.environ.get("MHA_MM_MODE", "bf16")
QB_OVERRIDE = int(os.environ.get("MHA_QB", "0"))        # 0 = mode default
ES_BUFS = int(os.environ.get("MHA_ES_BUFS", "8"))
KCG = int(os.environ.get("MHA_KCG", "2"))               # kc per exp group
SC_BUFS = int(os.environ.get("MHA_SC_BUFS", "2"))
PQ_BUFS = int(os.environ.get("MHA_PQ_BUFS", "2"))
Y_BF16 = bool(int(os.environ.get("MHA_Y_BF16", "1")))

FP32 = mybir.dt.float32
BF16 = mybir.dt.bfloat16
EXP = mybir.ActivationFunctionType.Exp


def _mode_params(mm_mode):
    if mm_mode == "bf16":
        dt, qb = mybir.dt.bfloat16, 512
    elif mm_mode == "f32r":
        dt, qb = mybir.dt.float32r, 512
    elif mm_mode == "f32":
        dt, qb = FP32, 256
    else:
        raise ValueError(mm_mode)
    return dt, (QB_OVERRIDE or qb)


def build_kernel(tc, xt, wqkv, bqkv, wout, y, sdt, QB, mm_mode, ctx):
    nc = tc.nc
    NQB = S // QB
    NG = NKC // KCG
    ydt = BF16 if Y_BF16 else FP32

    # float32r can only be produced by rounding-capable engine ops (ACT/DVE
    # outputs) or DMA of host-pre-rounded data; memset/affine_select cannot.
    vdt = FP32 if sdt == mybir.dt.float32r else sdt

    const = ctx.enter_context(tc.tile_pool(name="const", bufs=1))
    # PSUM budget: 8 banks = scores 2x2 + qkv/transpose/out-proj 2x1 + pa 2x1
    ps_sc = ctx.enter_context(tc.tile_pool(name="ps_sc", bufs=SC_BUFS,
                                           space="PSUM"))
    ps_q = ctx.enter_context(tc.tile_pool(name="ps_q", bufs=PQ_BUFS, space="PSUM"))
    ps_pa = ctx.enter_context(tc.tile_pool(name="ps_pa", bufs=2, space="PSUM"))

    xt_pool = ctx.enter_context(tc.tile_pool(name="xtp", bufs=3))
    exps_pool = ctx.enter_context(tc.tile_pool(name="exps", bufs=ES_BUFS))
    rc_pool = ctx.enter_context(tc.tile_pool(name="rc", bufs=2))
    bc_pool = ctx.enter_context(tc.tile_pool(name="bc", bufs=2))
    y_pool = ctx.enter_context(tc.tile_pool(name="yp", bufs=4))

    wq_r = wqkv.rearrange("(ko p) m -> p ko m", p=P)
    xt_r = xt.rearrange("(ko p) r -> p ko r", p=P)

    wq_sb = const.tile([P, KO, 3 * P], sdt)
    bq_sb = const.tile([P, 3], FP32)
    wo_sb = const.tile([P, E], sdt)
    ident = const.tile([P, P], vdt)

    qt = const.tile([P, B, S], sdt)       # Q^T  [2h*64, b, s]
    kt = const.tile([P, B, S], sdt)       # K^T
    vt = const.tile([P, B, S], vdt)       # V^T
    v1 = const.tile([P, B, HC, NKC, D + 1], sdt)  # V natural + ones col
    attnT = const.tile([P, B, S], sdt)    # unnormalized-then-normalized attn^T

    # ---- initial loads: wq/xt interleaved in fine chunks so the first QKV
    # matmuls start as soon as their slices land ----
    xt0 = xt_pool.tile([P, KO, RB], sdt, tag="xt", name="xt0")
    for ko in range(0, KO, 2):
        nc.sync.dma_start(wq_sb[:, ko:ko + 2, :], wq_r[:, ko:ko + 2, :])
        nc.sync.dma_start(xt0[:, ko:ko + 2, :], xt_r[:, ko:ko + 2, 0:RB])
        if ko == 0:
            nc.sync.dma_start(bq_sb, bqkv.rearrange("(m p) -> p m", p=P))
    nc.sync.dma_start(wo_sb, wout)

    make_identity(nc, ident)
    ones_col = const.tile([P, 1], FP32)
    nc.vector.memset(ones_col, 1.0)
    nc.vector.tensor_copy(v1[:, :, :, :, D:],
                          ones_col.to_broadcast((P, B, HC, NKC, 1)))
    ones_row = const.tile([1, D], vdt)
    nc.vector.memset(ones_row, 1.0)

    xt_tiles = {(0, 0): xt0}

    def emit_xt_dma(b, rbi):
        rb = b * NRB + rbi
        t = xt_pool.tile([P, KO, RB], sdt, tag="xt", name=f"xt_{rb}")
        nc.sync.dma_start(t, xt_r[:, :, rb * RB:(rb + 1) * RB])
        xt_tiles[(b, rbi)] = t

    def emit_mchunk(b, rbi, m):
        """One QKV dest (q/k/v) for one row block: 8 matmuls + bias add."""
        col = rbi * RB
        dest = (qt, kt, vt)[m]
        xt_t = xt_tiles[(b, rbi)]
        pst = ps_q.tile([P, RB], FP32, tag="pq", name=f"ps_qkv_{b}_{rbi}_{m}")
        for ko in range(KO):
            nc.tensor.matmul(
                pst, wq_sb[:, ko, m * P:(m + 1) * P], xt_t[:, ko, :],
                start=(ko == 0), stop=(ko == KO - 1))
        nc.vector.tensor_scalar_add(dest[:, b, col:col + RB], pst,
                                    bq_sb[:, m:m + 1])

    def emit_tr(b, rbi):
        """V transposes for one row block (into the qkv psum pool)."""
        for kci in range(RB // P):
            kc = rbi * (RB // P) + kci
            pst = ps_q.tile([P, P], vdt, tag="pq", name=f"ps_tr_{b}_{kc}")
            nc.tensor.transpose(pst, vt[:, b, kc * P:(kc + 1) * P], ident)
            nc.vector.tensor_copy(
                v1[:, b, :, kc, 0:D],
                pst.rearrange("p (h d) -> p h d", h=HC))

    def emit_outproj(b, qb, use_act=False, q_base=None, q_len=None):
        if q_base is None:
            q_base, q_len = qb * QB, QB
        for qc in range(q_len // P):
            q0 = q_base + qc * P
            yt = y_pool.tile([P, E], ydt, tag="yt", name=f"yt_{b}_{q_base}_{qc}")
            # on the final (drain) out-proj, alternate psum pools per qc to
            # double rotation depth; elsewhere ps_sc is owned by the scores
            pool, tg = (ps_q, "pq")
            if use_act and qc % 2 == 1:
                pool, tg = (ps_sc, "sc")
            for nh in range(2):
                pst = pool.tile([P, 512], FP32, tag=tg,
                                name=f"ps_y_{b}_{q_base}_{qc}_{nh}")
                nc.tensor.matmul(
                    pst, attnT[:, b, q0:q0 + P],
                    wo_sb[:, nh * 512:(nh + 1) * 512],
                    start=True, stop=True)
                if use_act and nh == 1:
                    # drain path: ACT is idle, split the psum->sbuf copies
                    nc.scalar.copy(yt[:, nh * 512:(nh + 1) * 512], pst)
                else:
                    nc.vector.tensor_copy(yt[:, nh * 512:(nh + 1) * 512], pst)
            nc.sync.dma_start(y[b * S + q0: b * S + q0 + P, :], yt)

    def emit_attention(b, qb, pre_hooks=None, post=None, fast_norm=False,
                       last=False, q_base=None, q_len=None, kcg=None):
        pre_hooks = pre_hooks or {}
        if q_base is None:
            q_base, q_len = qb * QB, QB
        kcg = kcg or KCG
        ng = NKC // kcg
        pa = {}
        for h in range(HC):
            pa[h] = ps_pa.tile([P, q_len], FP32, tag="pa",
                               name=f"pa_{b}_{qb}_{q_base}_{h}")
        def emit_pv(est, g):
            for h in range(HC):
                for j in range(kcg):
                    kc = g * kcg + j
                    nc.tensor.matmul(
                        pa[h][:D + 1, :], v1[:, b, h, kc, :],
                        est[h][:, j, :],
                        start=(kc == 0), stop=(kc == NKC - 1),
                        skip_group_check=True)

        # PV lags scores by one group so the first PV (which must wait for
        # the previous qb's normalize to release the pa bank) sits later in
        # the PE queue.
        pv_pending = None
        for g in range(ng):
            for fn in pre_hooks.get(g, ()):
                fn()
            est = {}
            pst = {}
            for h in range(HC):
                est[h] = exps_pool.tile([P, kcg, q_len], sdt, tag="es",
                                        name=f"es_{b}_{qb}_{q_base}_{g}_{h}")
                pst[h] = ps_sc.tile([P, kcg, q_len], FP32, tag="sc",
                                    name=f"ps_sc_{b}_{qb}_{q_base}_{g}_{h}")
            for h in range(HC):
                for j in range(kcg):
                    kc = g * kcg + j
                    nc.tensor.matmul(
                        pst[h][:, j, :],
                        kt[h * D:(h + 1) * D, b, kc * P:(kc + 1) * P],
                        qt[h * D:(h + 1) * D, b, q_base:q_base + q_len],
                        start=True, stop=True)
            for h in range(HC):
                nc.scalar.activation(est[h], pst[h], EXP, scale=0.125)
            if pv_pending is not None:
                emit_pv(*pv_pending)
            pv_pending = (est, g)
        emit_pv(*pv_pending)
        for h in range(HC):
            if fast_norm:
                # drain path: broadcast 1/den across partitions with a tiny
                # PE matmul (ones [1,D] stationary) — shorter latency than
                # the gpsimd partition_broadcast, and PE is idle here
                rc = rc_pool.tile([1, q_len], vdt, tag="rc",
                                  name=f"rc_{b}_{qb}_{q_base}_{h}")
                with nc.allow_low_precision("bf16 1/den on drain qb only"):
                    nc.vector.reciprocal(rc, pa[h][D:D + 1, :])
                bcp = ps_q.tile([D, q_len], FP32, tag="pq",
                                name=f"bcp_{b}_{qb}_{q_base}_{h}")
                nc.tensor.matmul(bcp, ones_row, rc, start=True, stop=True)
                nc.vector.tensor_tensor(
                    attnT[h * D:(h + 1) * D, b, q_base:q_base + q_len],
                    pa[h][:D, :], bcp, mybir.AluOpType.mult)
            elif not last:
                rc = rc_pool.tile([1, q_len], FP32, tag="rc",
                                  name=f"rc_{b}_{qb}_{q_base}_{h}")
                nc.vector.reciprocal(rc, pa[h][D:D + 1, :])
                # stage pa out of PSUM right away so the bank frees for the
                # next qb's PV before the broadcast/mult chain finishes
                sc = bc_pool.tile([D, q_len], FP32, tag="pascr",
                                  name=f"pasc_{b}_{qb}_{q_base}_{h}")
                nc.vector.tensor_copy(sc, pa[h][:D, :])
                bc = bc_pool.tile([D, q_len], FP32, tag="bc",
                                  name=f"bc_{b}_{qb}_{q_base}_{h}")
                nc.gpsimd.partition_broadcast(bc, rc)
                nc.vector.tensor_tensor(
                    attnT[h * D:(h + 1) * D, b, q_base:q_base + q_len],
                    sc, bc, mybir.AluOpType.mult)
            else:
                # final qb: no next PV to unblock — shortest chain to attnT
                rc = rc_pool.tile([1, q_len], FP32, tag="rc",
                                  name=f"rc_{b}_{qb}_{q_base}_{h}")
                nc.vector.reciprocal(rc, pa[h][D:D + 1, :])
                bc = bc_pool.tile([D, q_len], FP32, tag="bc",
                                  name=f"bc_{b}_{qb}_{q_base}_{h}")
                nc.gpsimd.partition_broadcast(bc, rc)
                nc.vector.tensor_tensor(
                    attnT[h * D:(h + 1) * D, b, q_base:q_base + q_len],
                    pa[h][:D, :], bc, mybir.AluOpType.mult)
        if post:
            post()

    def rb_hook(b, rbi, with_tr_of=None, dma_next=None):
        """pre_hooks dict entries that weave one row block of QKV through an
        attention qb: m-chunks after groups 1/3/5, transposes after 6."""
        hooks = {}
        if dma_next is not None:
            hooks.setdefault(0, []).append(
                lambda bn=dma_next[0], rn=dma_next[1]: emit_xt_dma(bn, rn))
        hooks.setdefault(1, []).append(lambda: emit_mchunk(b, rbi, 0))
        hooks.setdefault(3, []).append(lambda: emit_mchunk(b, rbi, 1))
        hooks.setdefault(5, []).append(lambda: emit_mchunk(b, rbi, 2))
        if with_tr_of is not None:
            hooks.setdefault(6, []).append(
                lambda r=with_tr_of: emit_tr(b, r))
        return hooks

    # ---- segment 1: QKV(b0) row blocks interleaved with attention(b0, 0) ----
    # rb0 must be fully emitted before group 0 (kt/qt/v1 for kc 0..3)
    emit_xt_dma(0, 1)
    for m in range(3):
        emit_mchunk(0, 0, m)
    seg1_hooks = {}
    seg1_hooks[0] = [lambda: emit_tr(0, 0), lambda: emit_xt_dma(0, 2)]
    seg1_hooks[1] = [lambda: (emit_mchunk(0, 1, 0), emit_mchunk(0, 1, 1),
                              emit_mchunk(0, 1, 2), emit_tr(0, 1),
                              emit_xt_dma(0, 3))]
    seg1_hooks[3] = [lambda: (emit_mchunk(0, 2, 0), emit_mchunk(0, 2, 1),
                              emit_mchunk(0, 2, 2), emit_tr(0, 2))]
    seg1_hooks[5] = [lambda: (emit_mchunk(0, 3, 0), emit_mchunk(0, 3, 1),
                              emit_mchunk(0, 3, 2), emit_tr(0, 3),
                              emit_xt_dma(1, 0))]
    emit_attention(0, 0, pre_hooks=seg1_hooks)

    # ---- segment 2: attention(b0, 1..3) carrying QKV(b1) rb 0..2 ----
    for qb in range(1, NQB):
        r = qb - 1
        hooks = rb_hook(1, r, with_tr_of=(r - 1) if r >= 1 else None,
                        dma_next=(1, r + 1) if r + 1 < NRB else None)
        emit_attention(0, qb, pre_hooks=hooks,
                       post=lambda q=qb - 1: emit_outproj(0, q))

    # ---- segment 3: attention(b1, 0..3), finishing QKV(b1) rb3 ----
    hooks = {
        1: [lambda: emit_mchunk(1, 3, 0)],
        3: [lambda: emit_mchunk(1, 3, 1)],
        4: [lambda: emit_tr(1, 2)],
        5: [lambda: emit_mchunk(1, 3, 2)],
        6: [lambda: emit_tr(1, 3)],
    }
    emit_attention(1, 0, pre_hooks=hooks, post=lambda: emit_outproj(0, 3))
    for qb in range(1, NQB - 1):
        emit_attention(1, qb, post=lambda q=qb - 1: emit_outproj(1, q))
    # final qb split into two 256-query halves (kcg=4 keeps the exp instr at
    # 1024 free elements, so ACT cost is unchanged); the first half's
    # out-proj fills the second half's window and the drain chain halves
    qb = NQB - 1
    emit_attention(1, qb, q_base=qb * QB, q_len=256, kcg=4,
                   post=lambda q=qb - 1: emit_outproj(1, q))
    emit_attention(1, qb, q_base=qb * QB + 256, q_len=256, kcg=4, last=True,
                   post=lambda: emit_outproj(1, qb, q_base=qb * QB,
                                             q_len=256))
    emit_outproj(1, qb, use_act=True, q_base=qb * QB + 256, q_len=256)


def build_nc(mm_mode=MM_MODE, reps=1):
    sdt, QB = _mode_params(mm_mode)
    ydt = BF16 if Y_BF16 else FP32
    nc = bacc.Bacc("TRN2", target_bir_lowering=False, debug=False)
    xt = nc.dram_tensor("xt", [E, R], sdt, kind="ExternalInput").ap()
    wqkv = nc.dram_tensor("wqkv", [E, 3 * P], sdt, kind="ExternalInput").ap()
    bqkv = nc.dram_tensor("bqkv", [3 * P], FP32, kind="ExternalInput").ap()
    wout = nc.dram_tensor("wout", [P, E], sdt, kind="ExternalInput").ap()
    y = nc.dram_tensor("y", [R, E], ydt, kind="ExternalOutput").ap()
    with tile.TileContext(nc) as tc:
        for _ in range(reps):
            with ExitStack() as ctx:
                build_kernel(tc, xt, wqkv, bqkv, wout, y, sdt, QB, mm_mode, ctx)
    nc.compile()
    return nc


def _round_f32r(x):
    """Round fp32 to the fp32r grid (11 explicit mantissa bits) the way the
    hardware expects matmul operands: add-half then truncate the low 12 bits."""
    bits = np.ascontiguousarray(x, np.float32).view(np.uint32)
    return (((bits + np.uint32(0x800)) & np.uint32(0xFFFFF000))
            .view(np.float32))


def shard_inputs(input_tensor, qkv_w, qkv_b, out_w, mm_mode=MM_MODE):
    """Build the 8 per-core input maps (numpy, host-side)."""
    sdt, _ = _mode_params(mm_mode)
    np_sdt = mybir.dt.np(sdt)

    def prep(a):
        a = np.ascontiguousarray(a).astype(np_sdt)
        return _round_f32r(a) if mm_mode == "f32r" else a

    X = np.asarray(input_tensor, np.float32).reshape(R, E)
    XT = prep(X.T)
    qkv_w = np.asarray(qkv_w, np.float32)
    qkv_b = np.asarray(qkv_b, np.float32)
    out_w = np.asarray(out_w, np.float32)
    in_maps = []
    for c in range(NCORES):
        sl = slice(c * P, (c + 1) * P)
        wq = np.concatenate(
            [qkv_w[:, sl], qkv_w[:, E + c * P:E + (c + 1) * P],
             qkv_w[:, 2 * E + c * P:2 * E + (c + 1) * P]], axis=1)
        bq = np.concatenate(
            [qkv_b[sl], qkv_b[E + c * P:E + (c + 1) * P],
             qkv_b[2 * E + c * P:2 * E + (c + 1) * P]])
        in_maps.append({
            "xt": XT,
            "wqkv": prep(wq),
            "bqkv": np.ascontiguousarray(bq),
            "wout": prep(out_w[sl, :]),
        })
    return in_maps


_NC_CACHE = {}


def _get_nc(mm_mode):
    if mm_mode not in _NC_CACHE:
        _NC_CACHE[mm_mode] = build_nc(mm_mode)
    return _NC_CACHE[mm_mode]


LAST_RESULT = None


def kernel(input_tensor, qkv_w, qkv_b, out_w, out_b):
    global LAST_RESULT
    from concourse import bass_utils
    nc = _get_nc(MM_MODE)
    in_maps = shard_inputs(input_tensor, qkv_w, qkv_b, out_w, MM_MODE)
    res = bass_utils.run_bass_kernel_spmd(
        nc, in_maps, core_ids=list(range(NCORES)),
        trace=bool(int(os.environ.get("MHA_TRACE", "0"))))
    LAST_RESULT = res
    out = np.zeros((R, E), np.float32)
    for r in res.results:
        out += np.asarray(r["y"], np.float32)
    out += np.asarray(out_b, np.float32)
    return out.reshape(B, S, E)


def core_partial_ref(input_tensor, qkv_w, qkv_b, out_w, c):
    """Exact fp32 numpy reference for core c's partial output (for testing)."""
    X = np.asarray(input_tensor, np.float32).reshape(R, E)
    out = np.zeros((R, E), np.float32)
    for b in range(B):
        rows = slice(b * S, (b + 1) * S)
        for hl in range(HC):
            h = c * HC + hl
            q = X[rows] @ qkv_w[:, h * D:(h + 1) * D] + qkv_b[h * D:(h + 1) * D]
            k = X[rows] @ qkv_w[:, E + h * D:E + (h + 1) * D] + qkv_b[E + h * D:E + (h + 1) * D]
            v = X[rows] @ qkv_w[:, 2 * E + h * D:2 * E + (h + 1) * D] + qkv_b[2 * E + h * D:2 * E + (h + 1) * D]
            s = (q @ k.T) / np.sqrt(np.float32(D))
            p = np.exp(s - s.max(axis=1, keepdims=True))
            p /= p.sum(axis=1, keepdims=True)
            a = p @ v
            out[rows] += a @ out_w[h * D:(h + 1) * D, :]
    return out


# revision 38
# speedup vs baseline: 1.4353x; 1.2714x over previous
"""Multi-head attention forward (B=2, S=2048, E=1024, H=16, D=64) on 8 TRN2
NeuronCores, tensor-parallel across heads (2 heads/core).

Per core: QKV^T projection with X^T streamed as the moving operand, attention
computed in the S^T/attn^T orientation (softmax denominator obtained by
appending a ones column to V in the PV matmul), out-projection of the core's
128 embed dims giving a partial [4096, 1024] output. Host sums the 8 partials
and adds the output bias.

Schedule: the attention inner loop is Activation-bound (exp), so all other
PE work is threaded through it: QKV row-blocks of batch b+1 and the
out-projection of the previous query block are emitted between score/PV
groups, attention(b0, qb0) starts as soon as the first QKV row block of b0
is done, and V transposes ride in the qkv/out-proj psum pool.
"""

import os
from contextlib import ExitStack

import numpy as np

import concourse.bass as bass
import concourse.mybir as mybir
import concourse.tile as tile
from concourse import bacc
from concourse.masks import make_identity

# ---- problem constants (hardcoded per contract) ----
B, S, E, H, D = 2, 2048, 1024, 16, 64
P = 128                      # partitions
R = B * S                    # 4096 flattened rows
KO = E // P                  # 8 contraction chunks over E
NKC = S // P                 # 16 key chunks per sequence
HC = 2                       # heads per core
NCORES = 8
RB = 512                     # row block for the QKV projection
NRB = S // RB                # row blocks per batch (4)

MM_MODE = os.environ.get("MHA_MM_MODE", "bf16")
QB_OVERRIDE = int(os.environ.get("MHA_QB", "0"))        # 0 = mode default
ES_BUFS = int(os.environ.get("MHA_ES_BUFS", "8"))
KCG = int(os.environ.get("MHA_KCG", "2"))               # kc per exp group
SC_BUFS = int(os.environ.get("MHA_SC_BUFS", "2"))
PQ_BUFS = int(os.environ.get("MHA_PQ_BUFS", "2"))
Y_BF16 = bool(int(os.environ.get("MHA_Y_BF16", "1")))

FP32 = mybir.dt.float32
BF16 = mybir.dt.bfloat16
EXP = mybir.ActivationFunctionType.Exp


def _mode_params(mm_mode):
    if mm_mode == "bf16":
        dt, qb = mybir.dt.bfloat16, 512
    elif mm_mode == "f32r":
        dt, qb = mybir.dt.float32r, 512
    elif mm_mode == "f32":
        dt, qb = FP32, 256
    else:
        raise ValueError(mm_mode)
    return dt, (QB_OVERRIDE or qb)


def build_kernel(tc, xt, wqkv, bqkv, wout, y, sdt, QB, mm_mode, ctx):
    nc = tc.nc
    NQB = S // QB
    NG = NKC // KCG
    ydt = BF16 if Y_BF16 else FP32

    # float32r can only be produced by rounding-capable engine ops (ACT/DVE
    # outputs) or DMA of host-pre-rounded data; memset/affine_select cannot.
    vdt = FP32 if sdt == mybir.dt.float32r else sdt

    const = ctx.enter_context(tc.tile_pool(name="const", bufs=1))
    # PSUM budget: 8 banks = scores 2x2 + qkv/transpose/out-proj 2x1 + pa 2x1
    ps_sc = ctx.enter_context(tc.tile_pool(name="ps_sc", bufs=SC_BUFS,
                                           space="PSUM"))
    ps_q = ctx.enter_context(tc.tile_pool(name="ps_q", bufs=PQ_BUFS, space="PSUM"))
    ps_pa = ctx.enter_context(tc.tile_pool(name="ps_pa", bufs=2, space="PSUM"))

    xt_pool = ctx.enter_context(tc.tile_pool(name="xtp", bufs=3))
    exps_pool = ctx.enter_context(tc.tile_pool(name="exps", bufs=ES_BUFS))
    rc_pool = ctx.enter_context(tc.tile_pool(name="rc", bufs=2))
    bc_pool = ctx.enter_context(tc.tile_pool(name="bc", bufs=2))
    y_pool = ctx.enter_context(tc.tile_pool(name="yp", bufs=4))

    wq_r = wqkv.rearrange("(ko p) m -> p ko m", p=P)
    xt_r = xt.rearrange("(ko p) r -> p ko r", p=P)

    wq_sb = const.tile([P, KO, 3 * P], sdt)
    bq_sb = const.tile([P, 3], FP32)
    wo_sb = const.tile([P, E], sdt)
    ident = const.tile([P, P], vdt)

    qt = const.tile([P, B, S], sdt)       # Q^T  [2h*64, b, s]
    kt = const.tile([P, B, S], sdt)       # K^T
    vt = const.tile([P, B, S], vdt)       # V^T
    v1 = const.tile([P, B, HC, NKC, D + 1], sdt)  # V natural + ones col
    attnT = const.tile([P, B, S], sdt)    # unnormalized-then-normalized attn^T

    # ---- initial loads: wq/xt interleaved in fine chunks so the first QKV
    # matmuls start as soon as their slices land ----
    xt0 = xt_pool.tile([P, KO, RB], sdt, tag="xt", name="xt0")
    for ko in range(0, KO, 2):
        nc.sync.dma_start(wq_sb[:, ko:ko + 2, :], wq_r[:, ko:ko + 2, :])
        nc.sync.dma_start(xt0[:, ko:ko + 2, :], xt_r[:, ko:ko + 2, 0:RB])
        if ko == 0:
            nc.sync.dma_start(bq_sb, bqkv.rearrange("(m p) -> p m", p=P))
    nc.sync.dma_start(wo_sb, wout)

    make_identity(nc, ident)
    ones_col = const.tile([P, 1], FP32)
    nc.vector.memset(ones_col, 1.0)
    nc.vector.tensor_copy(v1[:, :, :, :, D:],
                          ones_col.to_broadcast((P, B, HC, NKC, 1)))
    ones_row = const.tile([1, D], vdt)
    nc.vector.memset(ones_row, 1.0)

    xt_tiles = {(0, 0): xt0}

    def emit_xt_dma(b, rbi):
        rb = b * NRB + rbi
        t = xt_pool.tile([P, KO, RB], sdt, tag="xt", name=f"xt_{rb}")
        nc.sync.dma_start(t, xt_r[:, :, rb * RB:(rb + 1) * RB])
        xt_tiles[(b, rbi)] = t

    def emit_mchunk(b, rbi, m):
        """One QKV dest (q/k/v) for one row block: 8 matmuls + bias add."""
        col = rbi * RB
        dest = (qt, kt, vt)[m]
        xt_t = xt_tiles[(b, rbi)]
        pst = ps_q.tile([P, RB], FP32, tag="pq", name=f"ps_qkv_{b}_{rbi}_{m}")
        for ko in range(KO):
            nc.tensor.matmul(
                pst, wq_sb[:, ko, m * P:(m + 1) * P], xt_t[:, ko, :],
                start=(ko == 0), stop=(ko == KO - 1))
        nc.vector.tensor_scalar_add(dest[:, b, col:col + RB], pst,
                                    bq_sb[:, m:m + 1])

    def emit_tr(b, rbi):
        """V transposes for one row block (into the qkv psum pool)."""
        for kci in range(RB // P):
            kc = rbi * (RB // P) + kci
            pst = ps_q.tile([P, P], vdt, tag="pq", name=f"ps_tr_{b}_{kc}")
            nc.tensor.transpose(pst, vt[:, b, kc * P:(kc + 1) * P], ident)
            nc.vector.tensor_copy(
                v1[:, b, :, kc, 0:D],
                pst.rearrange("p (h d) -> p h d", h=HC))

    def emit_outproj(b, qb, use_act=False, q_base=None, q_len=None):
        if q_base is None:
            q_base, q_len = qb * QB, QB
        for qc in range(q_len // P):
            q0 = q_base + qc * P
            yt = y_pool.tile([P, E], ydt, tag="yt", name=f"yt_{b}_{q_base}_{qc}")
            # on the final (drain) out-proj, alternate psum pools per qc to
            # double rotation depth; elsewhere ps_sc is owned by the scores
            pool, tg = (ps_q, "pq")
            if use_act and qc % 2 == 1:
                pool, tg = (ps_sc, "sc")
            for nh in range(2):
                pst = pool.tile([P, 512], FP32, tag=tg,
                                name=f"ps_y_{b}_{q_base}_{qc}_{nh}")
                nc.tensor.matmul(
                    pst, attnT[:, b, q0:q0 + P],
                    wo_sb[:, nh * 512:(nh + 1) * 512],
                    start=True, stop=True)
                if use_act and nh == 1:
                    # drain path: ACT is idle, split the psum->sbuf copies
                    nc.scalar.copy(yt[:, nh * 512:(nh + 1) * 512], pst)
                else:
                    nc.vector.tensor_copy(yt[:, nh * 512:(nh + 1) * 512], pst)
            nc.sync.dma_start(y[b * S + q0: b * S + q0 + P, :], yt)

    def emit_attention(b, qb, pre_hooks=None, post=None, fast_norm=False,
                       last=False, q_base=None, q_len=None, kcg=None):
        pre_hooks = pre_hooks or {}
        if q_base is None:
            q_base, q_len = qb * QB, QB
        kcg = kcg or KCG
        ng = NKC // kcg
        pa = {}
        for h in range(HC):
            pa[h] = ps_pa.tile([P, q_len], FP32, tag="pa",
                               name=f"pa_{b}_{qb}_{q_base}_{h}")
        def emit_pv(est, g):
            for h in range(HC):
                for j in range(kcg):
                    kc = g * kcg + j
                    nc.tensor.matmul(
                        pa[h][:D + 1, :], v1[:, b, h, kc, :],
                        est[h][:, j, :],
                        start=(kc == 0), stop=(kc == NKC - 1),
                        skip_group_check=True)

        # PV lags scores by one group so the first PV (which must wait for
        # the previous qb's normalize to release the pa bank) sits later in
        # the PE queue.
        pv_pending = None
        for g in range(ng):
            for fn in pre_hooks.get(g, ()):
                fn()
            est = {}
            pst = {}
            for h in range(HC):
                est[h] = exps_pool.tile([P, kcg, q_len], sdt, tag="es",
                                        name=f"es_{b}_{qb}_{q_base}_{g}_{h}")
                pst[h] = ps_sc.tile([P, kcg, q_len], FP32, tag="sc",
                                    name=f"ps_sc_{b}_{qb}_{q_base}_{g}_{h}")
            for h in range(HC):
                for j in range(kcg):
                    kc = g * kcg + j
                    nc.tensor.matmul(
                        pst[h][:, j, :],
                        kt[h * D:(h + 1) * D, b, kc * P:(kc + 1) * P],
                        qt[h * D:(h + 1) * D, b, q_base:q_base + q_len],
                        start=True, stop=True)
            for h in range(HC):
                nc.scalar.activation(est[h], pst[h], EXP, scale=0.125)
            if pv_pending is not None:
                emit_pv(*pv_pending)
            pv_pending = (est, g)
        emit_pv(*pv_pending)
        for h in range(HC):
            if fast_norm:
                # drain path: broadcast 1/den across partitions with a tiny
                # PE matmul (ones [1,D] stationary) — shorter latency than
                # the gpsimd partition_broadcast, and PE is idle here
                rc = rc_pool.tile([1, q_len], vdt, tag="rc",
                                  name=f"rc_{b}_{qb}_{q_base}_{h}")
                with nc.allow_low_precision("bf16 1/den on drain qb only"):
                    nc.vector.reciprocal(rc, pa[h][D:D + 1, :])
                bcp = ps_q.tile([D, q_len], FP32, tag="pq",
                                name=f"bcp_{b}_{qb}_{q_base}_{h}")
                nc.tensor.matmul(bcp, ones_row, rc, start=True, stop=True)
                nc.vector.tensor_tensor(
                    attnT[h * D:(h + 1) * D, b, q_base:q_base + q_len],
                    pa[h][:D, :], bcp, mybir.AluOpType.mult)
            elif not last:
                rc = rc_pool.tile([1, q_len], FP32, tag="rc",
                                  name=f"rc_{b}_{qb}_{q_base}_{h}")
                nc.vector.reciprocal(rc, pa[h][D:D + 1, :])
                # stage pa out of PSUM right away so the bank frees for the
                # next qb's PV before the broadcast/mult chain finishes
                sc = bc_pool.tile([D, q_len], FP32, tag="pascr",
                                  name=f"pasc_{b}_{qb}_{q_base}_{h}")
                nc.vector.tensor_copy(sc, pa[h][:D, :])
                bc = bc_pool.tile([D, q_len], FP32, tag="bc",
                                  name=f"bc_{b}_{qb}_{q_base}_{h}")
                nc.gpsimd.partition_broadcast(bc, rc)
                nc.vector.tensor_tensor(
                    attnT[h * D:(h + 1) * D, b, q_base:q_base + q_len],
                    sc, bc, mybir.AluOpType.mult)
            else:
                # final qb: no next PV to unblock — shortest chain to attnT
                rc = rc_pool.tile([1, q_len], FP32, tag="rc",
                                  name=f"rc_{b}_{qb}_{q_base}_{h}")
                nc.vector.reciprocal(rc, pa[h][D:D + 1, :])
                bc = bc_pool.tile([D, q_len], FP32, tag="bc",
                                  name=f"bc_{b}_{qb}_{q_base}_{h}")
                nc.gpsimd.partition_broadcast(bc, rc)
                nc.vector.tensor_tensor(
                    attnT[h * D:(h + 1) * D, b, q_base:q_base + q_len],
                    pa[h][:D, :], bc, mybir.AluOpType.mult)
        if post:
            post()

    def rb_hook(b, rbi, with_tr_of=None, dma_next=None):
        """pre_hooks dict entries that weave one row block of QKV through an
        attention qb: m-chunks after groups 1/3/5, transposes after 6."""
        hooks = {}
        if dma_next is not None:
            hooks.setdefault(0, []).append(
                lambda bn=dma_next[0], rn=dma_next[1]: emit_xt_dma(bn, rn))
        hooks.setdefault(1, []).append(lambda: emit_mchunk(b, rbi, 0))
        hooks.setdefault(3, []).append(lambda: emit_mchunk(b, rbi, 1))
        hooks.setdefault(5, []).append(lambda: emit_mchunk(b, rbi, 2))
        if with_tr_of is not None:
            hooks.setdefault(6, []).append(
                lambda r=with_tr_of: emit_tr(b, r))
        return hooks

    # ---- segment 1: QKV(b0) row blocks interleaved with attention(b0, 0) ----
    # rb0 must be fully emitted before group 0 (kt/qt/v1 for kc 0..3)
    emit_xt_dma(0, 1)
    for m in range(3):
        emit_mchunk(0, 0, m)
    seg1_hooks = {}
    seg1_hooks[0] = [lambda: emit_tr(0, 0), lambda: emit_xt_dma(0, 2)]
    seg1_hooks[1] = [lambda: (emit_mchunk(0, 1, 0), emit_mchunk(0, 1, 1),
                              emit_mchunk(0, 1, 2), emit_tr(0, 1),
                              emit_xt_dma(0, 3))]
    seg1_hooks[3] = [lambda: (emit_mchunk(0, 2, 0), emit_mchunk(0, 2, 1),
                              emit_mchunk(0, 2, 2), emit_tr(0, 2))]
    seg1_hooks[5] = [lambda: (emit_mchunk(0, 3, 0), emit_mchunk(0, 3, 1),
                              emit_mchunk(0, 3, 2), emit_tr(0, 3),
                              emit_xt_dma(1, 0))]
    emit_attention(0, 0, pre_hooks=seg1_hooks)

    # ---- segment 2: attention(b0, 1..3) carrying QKV(b1) rb 0..2 ----
    for qb in range(1, NQB):
        r = qb - 1
        hooks = rb_hook(1, r, with_tr_of=(r - 1) if r >= 1 else None,
                        dma_next=(1, r + 1) if r + 1 < NRB else None)
        emit_attention(0, qb, pre_hooks=hooks,
                       post=lambda q=qb - 1: emit_outproj(0, q))

    # ---- segment 3: attention(b1, 0..3), finishing QKV(b1) rb3 ----
    hooks = {
        1: [lambda: emit_mchunk(1, 3, 0)],
        3: [lambda: emit_mchunk(1, 3, 1)],
        4: [lambda: emit_tr(1, 2)],
        5: [lambda: emit_mchunk(1, 3, 2)],
        6: [lambda: emit_tr(1, 3)],
    }
    emit_attention(1, 0, pre_hooks=hooks, post=lambda: emit_outproj(0, 3))
    for qb in range(1, NQB - 1):
        emit_attention(1, qb, post=lambda q=qb - 1: emit_outproj(1, q))
    # final qb split into two 256-query halves (kcg=4 keeps the exp instr at
    # 1024 free elements, so ACT cost is unchanged); the first half's
    # out-proj fills the second half's window and the drain chain halves
    qb = NQB - 1
    emit_attention(1, qb, q_base=qb * QB, q_len=256, kcg=4,
                   post=lambda q=qb - 1: emit_outproj(1, q))
    emit_attention(1, qb, q_base=qb * QB + 256, q_len=256, kcg=4, last=True,
                   post=lambda: emit_outproj(1, qb, q_base=qb * QB,
                                             q_len=256, use_act=True))
    emit_outproj(1, qb, use_act=True, q_base=qb * QB + 256, q_len=256)


def build_nc(mm_mode=MM_MODE, reps=1):
    sdt, QB = _mode_params(mm_mode)
    ydt = BF16 if Y_BF16 else FP32
    nc = bacc.Bacc("TRN2", target_bir_lowering=False, debug=False)
    xt = nc.dram_tensor("xt", [E, R], sdt, kind="ExternalInput").ap()
    wqkv = nc.dram_tensor("wqkv", [E, 3 * P], sdt, kind="ExternalInput").ap()
    bqkv = nc.dram_tensor("bqkv", [3 * P], FP32, kind="ExternalInput").ap()
    wout = nc.dram_tensor("wout", [P, E], sdt, kind="ExternalInput").ap()
    y = nc.dram_tensor("y", [R, E], ydt, kind="ExternalOutput").ap()
    with tile.TileContext(nc) as tc:
        for _ in range(reps):
            with ExitStack() as ctx:
                build_kernel(tc, xt, wqkv, bqkv, wout, y, sdt, QB, mm_mode, ctx)
    nc.compile()
    return nc


def _round_f32r(x):
    """Round fp32 to the fp32r grid (11 explicit mantissa bits) the way the
    hardware expects matmul operands: add-half then truncate the low 12 bits."""
    bits = np.ascontiguousarray(x, np.float32).view(np.uint32)
    return (((bits + np.uint32(0x800)) & np.uint32(0xFFFFF000))
            .view(np.float32))


def shard_inputs(input_tensor, qkv_w, qkv_b, out_w, mm_mode=MM_MODE):
    """Build the 8 per-core input maps (numpy, host-side)."""
    sdt, _ = _mode_params(mm_mode)
    np_sdt = mybir.dt.np(sdt)

    def prep(a):
        a = np.ascontiguousarray(a).astype(np_sdt)
        return _round_f32r(a) if mm_mode == "f32r" else a

    X = np.asarray(input_tensor, np.float32).reshape(R, E)
    XT = prep(X.T)
    qkv_w = np.asarray(qkv_w, np.float32)
    qkv_b = np.asarray(qkv_b, np.float32)
    out_w = np.asarray(out_w, np.float32)
    in_maps = []
    for c in range(NCORES):
        sl = slice(c * P, (c + 1) * P)
        wq = np.concatenate(
            [qkv_w[:, sl], qkv_w[:, E + c * P:E + (c + 1) * P],
             qkv_w[:, 2 * E + c * P:2 * E + (c + 1) * P]], axis=1)
        bq = np.concatenate(
            [qkv_b[sl], qkv_b[E + c * P:E + (c + 1) * P],
             qkv_b[2 * E + c * P:2 * E + (c + 1) * P]])
        in_maps.append({
            "xt": XT,
            "wqkv": prep(wq),
            "bqkv": np.ascontiguousarray(bq),
            "wout": prep(out_w[sl, :]),
        })
    return in_maps


_NC_CACHE = {}


def _get_nc(mm_mode):
    if mm_mode not in _NC_CACHE:
        _NC_CACHE[mm_mode] = build_nc(mm_mode)
    return _NC_CACHE[mm_mode]


LAST_RESULT = None


def kernel(input_tensor, qkv_w, qkv_b, out_w, out_b):
    global LAST_RESULT
    from concourse import bass_utils
    nc = _get_nc(MM_MODE)
    in_maps = shard_inputs(input_tensor, qkv_w, qkv_b, out_w, MM_MODE)
    res = bass_utils.run_bass_kernel_spmd(
        nc, in_maps, core_ids=list(range(NCORES)),
        trace=bool(int(os.environ.get("MHA_TRACE", "0"))))
    LAST_RESULT = res
    out = np.zeros((R, E), np.float32)
    for r in res.results:
        out += np.asarray(r["y"], np.float32)
    out += np.asarray(out_b, np.float32)
    return out.reshape(B, S, E)


def core_partial_ref(input_tensor, qkv_w, qkv_b, out_w, c):
    """Exact fp32 numpy reference for core c's partial output (for testing)."""
    X = np.asarray(input_tensor, np.float32).reshape(R, E)
    out = np.zeros((R, E), np.float32)
    for b in range(B):
        rows = slice(b * S, (b + 1) * S)
        for hl in range(HC):
            h = c * HC + hl
            q = X[rows] @ qkv_w[:, h * D:(h + 1) * D] + qkv_b[h * D:(h + 1) * D]
            k = X[rows] @ qkv_w[:, E + h * D:E + (h + 1) * D] + qkv_b[E + h * D:E + (h + 1) * D]
            v = X[rows] @ qkv_w[:, 2 * E + h * D:2 * E + (h + 1) * D] + qkv_b[2 * E + h * D:2 * E + (h + 1) * D]
            s = (q @ k.T) / np.sqrt(np.float32(D))
            p = np.exp(s - s.max(axis=1, keepdims=True))
            p /= p.sum(axis=1, keepdims=True)
            a = p @ v
            out[rows] += a @ out_w[h * D:(h + 1) * D, :]
    return out
